# revision 19
# baseline (speedup 1.0000x reference)
"""FBPINN PoU kernel for Trainium2 (8 NeuronCores).

Strategy
--------
The reference output is F(x[n]) for a single smooth scalar function
F: [0,1] -> R  (a partition-of-unity blend of 64 tiny tanh-MLPs times a
hard-BC factor tanh(5x)).  Because F is 1-D and C-infinity, we:

  1. host-side: sort the 65536 query points, shard contiguous slices of
     8192 sorted points to each of the 8 cores (data-parallel over N, as
     hinted — sorting is part of the sharding),
  2. on each core: evaluate F *exactly* (full f32 MLP + PoU formula) on a
     384-point slice of a global uniform grid with spacing h = 1/2048
     covering that core's x-range.  Gaussian PoU locality (sigma = 1.5/64)
     means only 28 subnet slots can contribute anywhere in a slice; the
     host packs exactly those subnets' weights for each core,
  3. on each core: build a 64-float-wide overlapped lookup table from the
     grid values, dma_gather the 4-point neighborhoods of all 8192 query
     points in one instruction, and evaluate the Catmull-Rom cubic
     interpolant on the vector engine,
  4. host-side: inverse-permute the per-core outputs back to input order.

Numerically validated: cubic interpolation at G=2048 reproduces the f32
reference to rel-norm ~1.3e-6 (the f32 round-off floor itself); the
7-spacing subnet window drops <5e-6 relative PoU weight.

Matmul packing: the per-subnet 64x64 matmuls are packed 4-up into the
128x128 PE array quadrants via tile_position; layer-0 outer products are
K=2 matmuls (x row + ones row) that fold the bias in for free.
"""

import os
import sys
import numpy as np

for _p in ("/opt/trn_rl_repo", "/root/.axon_site/_ro/trn_rl_repo"):
    if _p not in sys.path:
        sys.path.append(_p)

import concourse.bass as bass
import concourse.tile as tile
from concourse import bacc, mybir
from concourse.bass_utils import run_bass_kernel_spmd

# ---------------------------------------------------------------- constants
S = 64          # subdomains
W = 64          # MLP width
N = 65536       # query points
NCORES = 8
NPC = N // NCORES          # points per core = 8192
G = 2048                   # global uniform grid resolution (h = 1/G, exact in f32)
H = 1.0 / G
GS = 384                   # grid points per core slice (one matmul tile, N<=512)
GSP = GS + 64              # padded F table length in DRAM
SW = 28                    # subnet slots per core (window; 14 pairs, 7 quads)
NPAIR = SW // 2            # 14
NQUAD = SW // 4            # 7
SIGMA = 1.5 / S
DELTA = 1.0 / (S - 1)
MARGIN = 7.0               # PoU cutoff in units of DELTA
DEAD_CENTER = 10.0         # center for dead (padding) subnet slots

F32 = mybir.dt.float32
AF = mybir.ActivationFunctionType


# ---------------------------------------------------------------- device code
def _build_body(nc, tc, pools, aps, repeat):
    consts, hpool, ps, pst, dpool, gpool, ip = pools
    (rhs2, w0_sb, w1_sb, w2_sb, w3_sb, b1_sb, b2_sb, b3_sb, pou_sb,
     ones28, idx_sb, tv, yout) = aps
    no_gather = bool(os.environ.get("K_NO_GATHER"))

    for _rep in range(repeat):
        # ---- layer 0: h0 = tanh(W0*x + b0); K=2 matmuls fold the bias in
        h0 = []
        for p in range(NPAIR):
            r = 32 * (p % 4)
            pm = ps.tile([128, GS], F32, tag="mm", name="pm")
            nc.tensor.matmul(pm[:], w0_sb[r:r + 2, p, :], rhs2[r:r + 2, :],
                             tile_position=(r, 0))
            hp = hpool.tile([128, GS], F32, tag=f"h0_{p}", name="hp")
            nc.scalar.activation(hp[:], pm[:], AF.Tanh)
            h0.append(hp)

        # ---- layers 1 and 2: 4-up quadrant-packed 64x64 matmuls
        def mid_layer(hin, w_sb, b_sb, tagp):
            hA, hB = [], []
            for q in range(NQUAD):
                pa = ps.tile([128, GS], F32, tag="mm", name="pa")
                pb = ps.tile([128, GS], F32, tag="mm", name="pb")
                s0 = 4 * q
                if tagp == "h1":
                    # input pairing: (4q,4q+1), (4q+2,4q+3)  [natural h0 pairs]
                    nc.tensor.matmul(pa[0:64, :], w_sb[0:64, s0, :],
                                     hin[2 * q][0:64, :], tile_position=(0, 0))
                    nc.tensor.matmul(pb[0:64, :], w_sb[64:128, s0 + 1, :],
                                     hin[2 * q][64:128, :], tile_position=(64, 0))
                    nc.tensor.matmul(pb[64:128, :], w_sb[0:64, s0 + 2, :],
                                     hin[2 * q + 1][0:64, :], tile_position=(0, 64))
                    nc.tensor.matmul(pa[64:128, :], w_sb[64:128, s0 + 3, :],
                                     hin[2 * q + 1][64:128, :], tile_position=(64, 64))
                else:
                    # input pairing: A=(4q,4q+3), B=(4q+1 top, 4q+2 bottom)
                    nc.tensor.matmul(pa[0:64, :], w_sb[0:64, s0, :],
                                     hin[2 * q][0:64, :], tile_position=(0, 0))
                    nc.tensor.matmul(pa[64:128, :], w_sb[64:128, s0 + 3, :],
                                     hin[2 * q][64:128, :], tile_position=(64, 64))
                    nc.tensor.matmul(pb[0:64, :], w_sb[64:128, s0 + 2, :],
                                     hin[2 * q + 1][64:128, :], tile_position=(64, 0))
                    nc.tensor.matmul(pb[64:128, :], w_sb[0:64, s0 + 1, :],
                                     hin[2 * q + 1][0:64, :], tile_position=(0, 64))
                ha = hpool.tile([128, GS], F32, tag=f"{tagp}A_{q}", name="ha")
                nc.scalar.activation(ha[:], pa[:], AF.Tanh,
                                     bias=b_sb[:, 2 * q:2 * q + 1])
                hb = hpool.tile([128, GS], F32, tag=f"{tagp}B_{q}", name="hb")
                nc.scalar.activation(hb[:], pb[:], AF.Tanh,
                                     bias=b_sb[:, 2 * q + 1:2 * q + 2])
                hA.append(ha)
                hB.append(hb)
            out = []
            for q in range(NQUAD):
                out.append(hA[q])
                out.append(hB[q])
            return out

        h1 = mid_layer(h0, w1_sb, b1_sb, "h1")  # tiles: A=(4q,4q+3), B=(4q+1,4q+2)
        h2 = mid_layer(h1, w2_sb, b2_sb, "h2")  # tiles: A=(4q,4q+3), B=(4q+2,4q+1)

        # ---- layer 3: accumulate u[slot, n] via block lhsT, all pairs, 1 bank
        psU = pst.tile([SW, GS], F32, tag="psU", name="psU")
        for p in range(NPAIR):
            nc.tensor.matmul(psU[:], w3_sb[:, p, :], h2[p][:],
                             start=(p == 0), stop=(p == NPAIR - 1),
                             skip_group_check=True)
        us = gpool.tile([SW, GS], F32, tag="us", name="us")
        nc.scalar.activation(us[:], psU[:], AF.Identity, bias=b3_sb[:, 0:1])

        # ---- PoU weights: raw = exp(-0.5*((x-c)/sigma)^2)
        psZ = pst.tile([SW, GS], F32, tag="psZ", name="psZ")
        nc.tensor.matmul(psZ[:], pou_sb[:], rhs2[0:2, :])   # z = (x - c)/sigma
        sqz = gpool.tile([SW, GS], F32, tag="sqz", name="sqz")
        nc.scalar.activation(sqz[:], psZ[:], AF.Square)
        raw = gpool.tile([SW, GS], F32, tag="raw", name="raw")
        nc.scalar.activation(raw[:], sqz[:], AF.Exp, scale=-0.5)

        ru = gpool.tile([SW, GS], F32, tag="ru", name="ru")
        nc.vector.tensor_mul(ru[:], raw[:], us[:])

        psD = pst.tile([1, GS], F32, tag="psD", name="psD")
        nc.tensor.matmul(psD[:], ones28[:], raw[:])          # den
        psN = pst.tile([1, GS], F32, tag="psN", name="psN")
        nc.tensor.matmul(psN[:], ones28[:], ru[:])           # num

        t5 = gpool.tile([1, GS], F32, tag="t5", name="t5")
        nc.scalar.activation(t5[:], rhs2[0:1, :], AF.Tanh, scale=5.0)
        rden = gpool.tile([1, GS], F32, tag="rden", name="rden")
        nc.vector.reciprocal(rden[:], psD[:])
        fsb = gpool.tile([1, GSP], F32, tag="fsb", name="fsb")
        nc.vector.memset(fsb[:], 0.0)
        nc.vector.tensor_mul(fsb[0:1, 1:GS + 1], psN[:], rden[:])
        nc.vector.tensor_mul(fsb[0:1, 1:GS + 1], fsb[0:1, 1:GS + 1], t5[:])

        # ---- overlapped gather table in DRAM: T[r, k] = Fpad[r + k]
        fpad = dpool.tile([1, GSP], F32, tag="fpad", name="fpad")
        nc.sync.dma_start(fpad[:], fsb[:])
        tdram = dpool.tile([GS, 64], F32, tag="tdram", name="tdram")
        fpad_ovl = bass.AP(tensor=fpad[:].tensor, offset=fpad[:].offset,
                           ap=[[1, GS], [1, 64]])
        nc.sync.dma_start(tdram[:], fpad_ovl)

        # ---- gather 4-neighborhoods of all 8192 points (one instruction)
        g4 = gpool.tile([128, NPC // 128, 64], F32, tag="g4", name="g4")
        if no_gather:
            nc.vector.memset(g4[:], 0.0)
        else:
            nc.gpsimd.dma_gather(g4[:], tdram[:], idx_sb[:], NPC, NPC, 64,
                                 single_packet=False)

        # ---- Catmull-Rom cubic interpolation on DVE, [128, 64] layout
        PC = NPC // 128
        F0 = g4[:, :, 0]
        F1 = g4[:, :, 1]
        F2 = g4[:, :, 2]
        F3 = g4[:, :, 3]

        def tl(tag):
            return ip.tile([128, PC], F32, tag=tag, name=tag)

        STT = nc.vector.scalar_tensor_tensor
        MUL = mybir.AluOpType.mult
        ADD = mybir.AluOpType.add
        # y = F1 + 0.5*t*[(F2-F0) + t*[(2F0-5F1+4F2-F3) + t*(-F0+3F1-3F2+F3)]]
        e1 = tl("e1")
        nc.vector.tensor_sub(e1[:], F2, F0)
        f02 = tl("f02")
        nc.vector.tensor_add(f02[:], F0, F0)                     # 2*F0
        u1 = tl("u1")
        STT(u1[:], F1, -5.0, f02[:], op0=MUL, op1=ADD)           # -5F1 + 2F0
        f22 = tl("f22")
        nc.vector.tensor_add(f22[:], F2, F2)
        f24 = tl("f24")
        nc.vector.tensor_add(f24[:], f22[:], f22[:])             # 4*F2
        u2 = tl("u2")
        nc.vector.tensor_sub(u2[:], f24[:], F3)                  # 4F2 - F3
        e2 = tl("e2")
        nc.vector.tensor_add(e2[:], u1[:], u2[:])                # 2F0-5F1+4F2-F3
        d31 = tl("d31")
        nc.vector.tensor_sub(d31[:], F3, F0)
        d12 = tl("d12")
        nc.vector.tensor_sub(d12[:], F1, F2)
        e3 = tl("e3")
        STT(e3[:], d12[:], 3.0, d31[:], op0=MUL, op1=ADD)        # 3(F1-F2)+(F3-F0)
        acc = tl("acc")
        nc.vector.tensor_mul(acc[:], tv[:], e3[:])
        nc.vector.tensor_add(acc[:], acc[:], e2[:])
        nc.vector.tensor_mul(acc[:], acc[:], tv[:])
        nc.vector.tensor_add(acc[:], acc[:], e1[:])
        nc.vector.tensor_mul(acc[:], acc[:], tv[:])
        yt = tl("yt")
        STT(yt[:], acc[:], 0.5, F1, op0=MUL, op1=ADD)
        nc.sync.dma_start(yout, yt[:])


def _build_program(repeat=1):
    nc = bacc.Bacc("TRN2", target_bir_lowering=False, debug=False,
                   num_devices=NCORES)

    def din(name, shape, dtype=F32):
        return nc.dram_tensor(name, list(shape), dtype, kind="ExternalInput").ap()

    gxv = din("gxv", [1, GS])                  # grid x coords
    W0b = din("W0b", [128, NPAIR, 128])        # L0 lhsT, replicated rows {0,32,64,96}
    W1T = din("W1T", [128, SW, W])             # W1[s].T, rows [0:64]==[64:128]
    W2T = din("W2T", [128, SW, W])
    W3blk = din("W3blk", [128, NPAIR, SW])     # L3 block lhsT per pair-tile
    b1p = din("b1p", [128, NPAIR])
    b2p = din("b2p", [128, NPAIR])
    b3v = din("b3v", [SW, 1])
    pouL = din("pouL", [2, SW])                # rows: [1/sigma ; -c_j/sigma]
    idx16 = din("idx16", [128, NPC // 16], mybir.dt.int16)
    tval = din("tval", [128, NPC // 128])
    yout = nc.dram_tensor("y", [128, NPC // 128], F32, kind="ExternalOutput").ap()

    with tile.TileContext(nc) as tc:
        consts = tc.alloc_tile_pool(name="consts", bufs=1)
        hpool = tc.alloc_tile_pool(name="hpool", bufs=1)
        ps = tc.alloc_tile_pool(name="ps", bufs=4, space="PSUM")
        pst = tc.alloc_tile_pool(name="pst", bufs=1, space="PSUM")
        dpool = tc.alloc_tile_pool(name="dpool", bufs=1, space="DRAM")
        gpool = tc.alloc_tile_pool(name="gpool", bufs=1)
        ip = tc.alloc_tile_pool(name="interp", bufs=1)

        # ---- load constants into SBUF
        # [gx ; ones] at partition pairs {0,32,64,96} so 4-up row-packed L0
        # matmuls see fmap and weights at the same base partition
        rhs2 = consts.tile([128, GS], F32, tag="rhs2", name="rhs2")
        nc.vector.memset(rhs2[:], 1.0)
        for r in (0, 32, 64, 96):
            nc.sync.dma_start(rhs2[r:r + 1, :], gxv)

        w0_sb = consts.tile([128, NPAIR, 128], F32, tag="w0", name="w0_sb")
        nc.sync.dma_start(w0_sb[:], W0b)
        w1_sb = consts.tile([128, SW, W], F32, tag="w1", name="w1_sb")
        nc.sync.dma_start(w1_sb[:], W1T)
        w2_sb = consts.tile([128, SW, W], F32, tag="w2", name="w2_sb")
        nc.sync.dma_start(w2_sb[:], W2T)
        w3_sb = consts.tile([128, NPAIR, SW], F32, tag="w3", name="w3_sb")
        nc.sync.dma_start(w3_sb[:], W3blk)
        b1_sb = consts.tile([128, NPAIR], F32, tag="b1", name="b1_sb")
        nc.sync.dma_start(b1_sb[:], b1p)
        b2_sb = consts.tile([128, NPAIR], F32, tag="b2", name="b2_sb")
        nc.sync.dma_start(b2_sb[:], b2p)
        b3_sb = consts.tile([SW, 1], F32, tag="b3", name="b3_sb")
        nc.sync.dma_start(b3_sb[:], b3v)
        pou_sb = consts.tile([2, SW], F32, tag="pou", name="pou_sb")
        nc.sync.dma_start(pou_sb[:], pouL)
        ones28 = consts.tile([SW, 1], F32, tag="ones28", name="ones28")
        nc.vector.memset(ones28[:], 1.0)
        idx_sb = consts.tile([128, NPC // 16], mybir.dt.int16, tag="idx",
                             name="idx_sb")
        nc.sync.dma_start(idx_sb[:], idx16)
        tv = consts.tile([128, NPC // 128], F32, tag="tv", name="tv")
        nc.sync.dma_start(tv[:], tval)

        pools = (consts, hpool, ps, pst, dpool, gpool, ip)
        aps = (rhs2, w0_sb, w1_sb, w2_sb, w3_sb, b1_sb, b2_sb, b3_sb,
               pou_sb, ones28, idx_sb, tv, yout)
        _build_body(nc, tc, pools, aps, repeat)

        for pool in (ip, gpool, dpool, pst, ps, hpool, consts):
            pool.release()

    nc.compile()
    return nc


_PROGRAMS = {}


def _get_program(repeat=1):
    if repeat not in _PROGRAMS:
        _PROGRAMS[repeat] = _build_program(repeat)
    return _PROGRAMS[repeat]


# ---------------------------------------------------------------- host prep
def prepare_core_inputs(xs_c, W0, b0, W1, b1, W2, b2, W3, b3):
    """Build the per-core input map for one slice of sorted points.

    xs_c: [NPC] float64 sorted x values for this core.
    """
    i_cell = np.floor(xs_c * G).astype(np.int64)
    tfrac = (xs_c * G - i_cell).astype(np.float32)
    g0 = int(i_cell.min()) - 1
    span = int(i_cell.max()) - g0
    assert span <= GS - 3, f"grid slice overflow: span={span}"
    li = (i_cell - g0).astype(np.int16)            # gather row, in [1, GS-3]
    assert li.min() >= 1 and li.max() <= GS - 3

    gx = ((g0 + np.arange(GS)) * H).astype(np.float32)

    # subnet slot selection
    centers = np.linspace(0.0, 1.0, S)
    lo, hi = gx[0] - MARGIN * DELTA, gx[-1] + MARGIN * DELTA
    need = np.where((centers >= lo) & (centers <= hi))[0]
    assert len(need) <= SW, f"subnet window overflow: {len(need)}"
    s0 = int(need[0])
    slots = s0 + np.arange(SW)                      # may run past S-1 -> dead
    alive = (slots < S) & np.isin(slots, need)

    def wslot(arr, j, default=0.0):
        return arr[slots[j]] if alive[j] else np.full(arr.shape[1:], default,
                                                      arr.dtype)

    # L0 lhsT [128, NPAIR, 128], replicated at partition rows {0,32,64,96}
    W0b = np.zeros((128, NPAIR, 128), np.float32)
    for p in range(NPAIR):
        a, b = 2 * p, 2 * p + 1
        row0 = np.concatenate([wslot(W0, a), wslot(W0, b)])     # [128]
        row1 = np.concatenate([wslot(b0, a), wslot(b0, b)])
        for r in (0, 32, 64, 96):
            W0b[r, p, :] = row0
            W0b[r + 1, p, :] = row1

    # W1T/W2T [128, SW, 64]: W[s].T duplicated on both partition halves
    def midT(Wl):
        out = np.zeros((128, SW, W), np.float32)
        for j in range(SW):
            wt = wslot(Wl, j).T                                  # [w_in, v]
            out[0:64, j, :] = wt
            out[64:128, j, :] = wt
        return out

    W1Tb = midT(W1)
    W2Tb = midT(W2)

    # biases in pair-tile layout
    b1p = np.zeros((128, NPAIR), np.float32)
    b2p = np.zeros((128, NPAIR), np.float32)
    for q in range(NQUAD):
        s_ = 4 * q
        b1p[0:64, 2 * q] = wslot(b1, s_)
        b1p[64:128, 2 * q] = wslot(b1, s_ + 3)
        b1p[0:64, 2 * q + 1] = wslot(b1, s_ + 1)
        b1p[64:128, 2 * q + 1] = wslot(b1, s_ + 2)
        b2p[0:64, 2 * q] = wslot(b2, s_)
        b2p[64:128, 2 * q] = wslot(b2, s_ + 3)
        b2p[0:64, 2 * q + 1] = wslot(b2, s_ + 2)
        b2p[64:128, 2 * q + 1] = wslot(b2, s_ + 1)

    # L3 block lhsT: pair-tile 2q = (4q top, 4q+3 bottom); 2q+1 = (4q+2, 4q+1)
    W3blk = np.zeros((128, NPAIR, SW), np.float32)
    for q in range(NQUAD):
        s_ = 4 * q
        W3blk[0:64, 2 * q, s_] = wslot(W3, s_)
        W3blk[64:128, 2 * q, s_ + 3] = wslot(W3, s_ + 3)
        W3blk[0:64, 2 * q + 1, s_ + 2] = wslot(W3, s_ + 2)
        W3blk[64:128, 2 * q + 1, s_ + 1] = wslot(W3, s_ + 1)

    b3v = np.array([[wslot(b3, j)] for j in range(SW)],
                   np.float32).reshape(SW, 1)

    pouLm = np.zeros((2, SW), np.float32)
    pouLm[0, :] = 1.0 / SIGMA
    for j in range(SW):
        c = centers[slots[j]] if alive[j] else DEAD_CENTER
        pouLm[1, j] = -c / SIGMA

    idx_w = np.tile(li.reshape(NPC // 16, 16).T, (8, 1)).astype(np.int16)
    t_arr = tfrac.reshape(NPC // 128, 128).T.copy()

    return {
        "gxv": gx.reshape(1, GS),
        "W0b": W0b, "W1T": W1Tb, "W2T": W2Tb, "W3blk": W3blk,
        "b1p": b1p, "b2p": b2p, "b3v": b3v, "pouL": pouLm,
        "idx16": idx_w, "tval": t_arr,
    }


def host_prepare(inputs):
    x = np.asarray(inputs["x"], np.float32)[:, 0]
    perm = np.argsort(x, kind="stable")
    xs = x[perm].astype(np.float64)
    W0 = np.asarray(inputs["W0"], np.float32)[:, :, 0]
    b0 = np.asarray(inputs["b0"], np.float32)
    W1 = np.asarray(inputs["W1"], np.float32)
    b1 = np.asarray(inputs["b1"], np.float32)
    W2 = np.asarray(inputs["W2"], np.float32)
    b2 = np.asarray(inputs["b2"], np.float32)
    W3 = np.asarray(inputs["W3"], np.float32)[:, 0, :]
    b3 = np.asarray(inputs["b3"], np.float32)[:, 0]
    in_maps = []
    for c in range(NCORES):
        xs_c = xs[c * NPC:(c + 1) * NPC]
        in_maps.append(prepare_core_inputs(xs_c, W0, b0, W1, b1, W2, b2,
                                           W3, b3))
    return perm, in_maps


def unshard(perm, core_outs):
    y = np.empty(N, np.float32)
    for c in range(NCORES):
        yc = np.asarray(core_outs[c])                 # [128, NPC//128]
        y[perm[c * NPC:(c + 1) * NPC]] = yc.T.reshape(-1)
    return y.reshape(N, 1)


# ---------------------------------------------------------------- entry point
def kernel(**inputs):
    perm, in_maps = host_prepare(inputs)
    nc = _get_program()
    res = run_bass_kernel_spmd(nc, in_maps, list(range(NCORES)))
    return unshard(perm, [r["y"] for r in res.results])
